# revision 17
# baseline (speedup 1.0000x reference)
"""Causal multi-head attention (B=4, S=2048, D=1024, H=16) on 8 NeuronCores.

Sharding: core c handles batch b=c//2 and head-group g=c%2 (8 heads, 512
features). The host pre-transposes x and the weight slices so every device
matmul contracts along the partition dim; the row-parallel out-projection
partials are summed pairwise on the host (+ bias).

Per-core pipeline (one Bass/Tile program, SPMD over 8 cores):
  1. qT/kT feature-major and v token-major via bf16 matmuls at full PE rate
     (bf16 inputs halve the HBM traffic; layer-chunked DMAs in d-layer order
     overlap the dk-major sweeps, critical first chunks issued first, wo
     deferred to last).  v is stored as vtm[p, nt, h, 65] with a ones column
     at slot 64 (the softmax-denominator trick).
  2. Causal flash attention, two heads per stream (K=64 scores at PE tile
     positions 0/64), q-blocks of 512, k-tiles of 128 grouped 3 per PSUM
     tile; exp on ACT (scale 1/8) straight out of PSUM into bf16;
     triangular diagonal tiles masked multiplicatively; fully-masked tiles
     never computed.  Context accumulates as ctx_augT[65,512] whose row 64
     is the denominator.  A one-group software pipeline is carried across
     (qb, pair) iterations.
  3. Softmax normalization deferred: per-qb denominator rows packed by one
     DMA into a [64,64] block for a cheap vector reciprocal, split hi/lo
     into bf16, and broadcast with K=2 PE outer products; ctxT (bf16)
     written by one fused multiply per (pair, q-block).
  4. out_partial[2048,1024] = ctxT.T @ Wo.T-slice (bf16) per 512-token
     range; bf16 out partials summed in f32 on the host (+ bias).
"""

import sys
import types

import numpy as np
import ml_dtypes

import concourse.bass as bass
import concourse.mybir as mybir
from concourse import tile
from concourse.bass_utils import run_bass_kernel_spmd
from concourse.masks import make_upper_triangular

# ----------------------------------------------------------------------------
# Compat patches for this container (self-contained on purpose).
# ----------------------------------------------------------------------------


def _patch_tail_drain():
    """This walrus build accepts only ONE sync-wait per sync-engine
    instruction; TileContext's tail drain may carry several. Split extras
    onto dedicated 1-wait nops."""
    from concourse.vector_clock import ScopedClock

    def _drain_and_barrier(self, tick_clock, wait_clock):
        nc = self.nc
        drain_inst = nc.sync.drain()
        wait_clock.add_sem_waits(
            drain_inst.ins, ScopedClock({None: tick_clock.global_clock})
        )
        si = drain_inst.ins.sync_info
        if si is not None and len(si.on_wait) > 1:
            waits = list(si.on_wait)
            drain_inst.ins.sync_info = mybir.SyncInfo(
                on_wait=waits[:1], on_update=list(si.on_update)
            )
            for w in waits[1:]:
                n = nc.sync.nop()
                n.ins.sync_info = mybir.SyncInfo(on_wait=[w], on_update=[])

        nc.all_engine_barrier()
        assert self.sems is not None
        popped = nc._tile_sem_poison_stack.pop()
        assert popped is self._sem_poison
        nc.clear_and_free_semaphores(list(self.sems.allocated().values()))
        nc.all_engine_barrier()

    tile.TileContext._drain_and_barrier = _drain_and_barrier


def _patch_profiling():
    """Provide the NTFF profile hook (image's antenv lacks axon_hooks) and
    disable cloud artifact uploads. Only matters when tracing is requested."""
    import concourse.bass_utils as bass_utils

    bass_utils.upload_artifacts = lambda tmpdir: tmpdir
    try:
        from antenv.axon_hooks import get_axon_ntff_profile_hook  # noqa: F401
        return
    except ImportError:
        pass
    try:
        from trn_agent_boot.trn_boot import _ntff_profile_via_ctypes

        hook = _ntff_profile_via_ctypes("/opt/axon/libaxon_pjrt.so")
    except Exception:
        hook = None
    mod = types.ModuleType("antenv.axon_hooks")
    mod._hook = hook
    mod.get_axon_ntff_profile_hook = lambda: mod._hook
    mod.set_axon_ntff_profile_hook = lambda h: setattr(mod, "_hook", h)
    sys.modules["antenv.axon_hooks"] = mod
    import antenv

    antenv.axon_hooks = mod


_patch_tail_drain()
_patch_profiling()


def _legalize_waits(nc):
    """This walrus build allows 1 sync-wait per instruction (2 on
    EventSemaphore). Split excess waits onto EventSemaphore carriers
    inserted just before the over-capacity instruction (same engine
    queue, so ordering semantics are preserved)."""
    n_fix = 0
    for f in nc.m.functions:
        for b in f.blocks:
            out = []
            changed = False
            for inst in b.instructions:
                si = inst.sync_info
                cap = 1
                if si is not None and len(si.on_wait) > cap:
                    waits = list(si.on_wait)
                    extra, keep = waits[:-cap], waits[-cap:]
                    for i in range(0, len(extra), 1):
                        n_fix += 1
                        out.append(
                            mybir.InstNoOp(
                                name=f"I-waitfix-{n_fix}",
                                engine=inst.engine,
                                ins=[],
                                outs=[],
                                sync_info=mybir.SyncInfo(
                                    on_wait=extra[i:i + 1], on_update=[]
                                ),
                            )
                        )
                    inst.sync_info = mybir.SyncInfo(
                        on_wait=keep, on_update=list(si.on_update)
                    )
                    changed = True
                out.append(inst)
            if changed:
                b.instructions = out

# ----------------------------------------------------------------------------
# Problem constants (hardcoded; kernel.py must be self-contained).
# ----------------------------------------------------------------------------
B, S, D, H = 4, 2048, 1024, 16
HD = D // H          # 64 head dim
NCORES = 8
GPC = 2              # head-groups per batch (cores per batch)
FPC = D // GPC       # 512 features per core
HPC = H // GPC       # 8 heads per core
P = 128
DC = D // P          # 8 contraction chunks
NT = S // P          # 16 token tiles
QB = 512             # q-block
NQB = S // QB        # 4

F32 = mybir.dt.float32
F32R = mybir.dt.float32r
BF16 = mybir.dt.bfloat16
FP8 = mybir.dt.float8e4
DR = mybir.MatmulPerfMode.DoubleRow
EXPF = mybir.ActivationFunctionType.Exp
SCALE = 1.0 / np.sqrt(HD)


def _build_program():
    nc = bass.Bass("TRN2", target_bir_lowering=False, debug=False, num_devices=1)
    xT = nc.dram_tensor("xT", [D, S], BF16, kind="ExternalInput").ap()
    wq = nc.dram_tensor("wq", [D, FPC], BF16, kind="ExternalInput").ap()
    wk = nc.dram_tensor("wk", [D, FPC], BF16, kind="ExternalInput").ap()
    wv = nc.dram_tensor("wv", [D, FPC], BF16, kind="ExternalInput").ap()
    wo = nc.dram_tensor("wo", [FPC, D], BF16, kind="ExternalInput").ap()
    out = nc.dram_tensor("out", [S, D], BF16, kind="ExternalOutput").ap()

    with tile.TileContext(nc) as tc:
        _emit(nc, tc, xT, wq, wk, wv, wo, out)
    _legalize_waits(nc)
    return nc


def _emit(nc, tc, xT, wq, wk, wv, wo, out):
    persist = tc.alloc_tile_pool(name="persist", bufs=1)

    qT = persist.tile([P, NQB, S], BF16, tag="qT")
    kT = persist.tile([P, NQB, S], BF16, tag="kT")
    vtm = persist.tile([P, NT, HPC, HD + 1], BF16, tag="vtm")
    ctxT = persist.tile([P, NQB, S], BF16, tag="ctxT")
    wo_sb = persist.tile([P, FPC // P, D], BF16, tag="wo_sb")
    dmask_f = persist.tile([P, P], F32, tag="dmask_f")
    dmask = persist.tile([P, P], BF16, tag="dmask")

    # one-time setup (wo DMA deferred to after wv below: it is not needed
    # until the first out-projection, and it must not delay the x/w chase)
    make_upper_triangular(nc, dmask_f[:], val=1.0, diag=True)
    nc.vector.tensor_copy(dmask[:], dmask_f[:])
    nc.vector.memset(vtm[:, :, :, HD], 1.0)  # denominator ones column

    # ---------------- Phase A: QKV projections (bf16) ----------------
    # Chunked DMAs in d-layer order so the dk-major matmul sweeps can start
    # as soon as the first layers land instead of waiting for the full 8 MB.
    with (
        tc.tile_pool(name="loadA", bufs=1) as la,
        tc.tile_pool(name="psA", bufs=8, space="PSUM") as psA,
    ):
        xT_sb = la.tile([P, DC, S], BF16, tag="xT_sb")
        wq_sb = la.tile([P, DC, FPC], BF16, tag="wq_sb")
        wk_sb = la.tile([P, DC, FPC], BF16, tag="wk_sb")
        wv_sb = la.tile([P, DC, FPC], BF16, tag="wv_sb")
        nc.sync.dma_start(wq_sb[:, 0, :], wq[0:P, :])
        nc.sync.dma_start(xT_sb[:, 0, :], xT[0:P, :])
        nc.sync.dma_start(wk_sb[:, 0, :], wk[0:P, :])
        for dkk in range(1, DC, 2):
            hi = min(dkk + 2, DC)
            r = slice(dkk * P, hi * P)
            nc.sync.dma_start(
                wq_sb[:, dkk:hi, :],
                wq[r, :].rearrange("(c p) e -> p c e", p=P),
            )
            nc.sync.dma_start(
                wk_sb[:, dkk:hi, :],
                wk[r, :].rearrange("(c p) e -> p c e", p=P),
            )
            for dk in range(dkk, hi):
                nc.sync.dma_start(xT_sb[:, dk, :], xT[dk * P:(dk + 1) * P, :])
        nc.sync.dma_start(wv_sb[:], wv.rearrange("(c p) e -> p c e", p=P))
        nc.sync.dma_start(wo_sb[:], wo.rearrange("(c p) e -> p c e", p=P))

        # q/k in m-tile PAIRS, dk-middle, nb-inner: 8 live psums, one
        # weight load per 4 matmuls, and the DMA layer-chase overlaps two
        # blocks' worth of compute.
        for pj, w_sb, dst in ((0, wq_sb, qT), (1, wk_sb, kT)):
            for m0 in range(0, FPC // P, 2):
                pss = {}
                for mi in range(2):
                    for nb in range(NQB):
                        pss[(mi, nb)] = psA.tile(
                            [P, QB], F32, tag="psA", name=f"psA{mi}_{nb}"
                        )
                for dk in range(DC):
                    for mi in range(2):
                        m = m0 + mi
                        for nb in range(NQB):
                            nc.tensor.matmul(
                                pss[(mi, nb)][:],
                                lhsT=w_sb[:, dk, m * P:(m + 1) * P],
                                rhs=xT_sb[:, dk, nb * QB:(nb + 1) * QB],
                                start=(dk == 0),
                                stop=(dk == DC - 1),
                            )
                for mi in range(2):
                    for nb in range(NQB):
                        nc.vector.tensor_copy(
                            dst[:, m0 + mi, nb * QB:(nb + 1) * QB],
                            pss[(mi, nb)][:],
                        )

        for nt in range(NT):
            ps = psA.tile([P, FPC], F32, tag="psA")
            for dk in range(DC):
                nc.tensor.matmul(
                    ps[:],
                    lhsT=xT_sb[:, dk, nt * P:(nt + 1) * P],
                    rhs=wv_sb[:, dk, :],
                    start=(dk == 0),
                    stop=(dk == DC - 1),
                )
            nc.vector.tensor_copy(
                vtm[:, nt, :, 0:HD],
                ps[:].rearrange("p (h d) -> p h d", h=HPC),
            )

    # ---------------- Phase B: causal attention ----------------
    # Paired-head streams (row-packed K=64 scores), qb-outer order, one-group
    # software pipeline carried ACROSS (qb, pair) iterations so ACT never
    # stalls at boundaries. Unnormalized ctx staged f32 in ctxU; denominator
    # rows staged into denstage (row idx = qb*8+h); per-qb reciprocal +
    # bf16 hi/lo split + rrowbig shipping run during later attention work.
    stageB = tc.alloc_tile_pool(name="stageB", bufs=1)
    ctxU = stageB.tile([P, NQB, S], F32, tag="ctxU")
    dstage = stageB.tile([P, HPC, QB], F32, tag="dstage")  # row HD, slot h
    den2 = stageB.tile([P, NQB, HD], F32, tag="den2")  # [h*8+qhi, qb, qlo]
    rec2 = stageB.tile([P, NQB, HD], F32, tag="rec2")
    hilo2 = stageB.tile([P, NQB, 2, HD], BF16, tag="hilo2")
    e1t2 = stageB.tile([P, NQB, HD], F32, tag="e1t2")
    onesbf = stageB.tile([P, HD], BF16, tag="onesbf")
    rrowbig = stageB.tile([P, HPC * NQB, QB], BF16, tag="rrowbig")
    nc.vector.memset(onesbf[:], 1.0)

    OFFS = (0, 512, 1024, 1280)
    LENS = (512, 384, 256, 128)
    QOFFS = (0, 128, 256, 384)

    with (
        tc.tile_pool(name="expp", bufs=6) as expp,
        tc.tile_pool(name="scps", bufs=1, space="PSUM") as scps,
        tc.tile_pool(name="ctxps", bufs=1, space="PSUM") as ctxps,
        tc.tile_pool(name="outsb", bufs=3) as outsb,
    ):
        def emit_scores_exp(m2, qb, heads, kind, kts):
            es, pss = {}, {}
            for i, hh in enumerate(heads):
                pss[hh] = scps.tile([P, 3 * QB], F32, tag=f"sc{i}", name=f"sc{i}")
                es[hh] = expp.tile([P, 3 * QB], BF16, tag=f"es{i}", name=f"es{i}")
            if kind == "full":
                n = len(kts) * QB
                for hh in heads:
                    hp = (hh % 2) * HD
                    for i, kt in enumerate(kts):
                        nc.tensor.matmul(
                            pss[hh][:, i * QB:(i + 1) * QB],
                            lhsT=kT[hp:hp + HD, m2, kt * P:(kt + 1) * P],
                            rhs=qT[hp:hp + HD, m2, qb * QB:(qb + 1) * QB],
                            start=True,
                            stop=True,
                        )
                    nc.scalar.activation(
                        es[hh][:, 0:n], pss[hh][:, 0:n], EXPF, scale=SCALE
                    )
            else:
                for hh in heads:
                    hp = (hh % 2) * HD
                    for j in range(4):
                        nc.tensor.matmul(
                            pss[hh][:, OFFS[j]:OFFS[j] + LENS[j]],
                            lhsT=kT[hp:hp + HD, m2, kts[j] * P:(kts[j] + 1) * P],
                            rhs=qT[hp:hp + HD, m2,
                                   qb * QB + QOFFS[j]:(qb + 1) * QB],
                            start=(j != 3),
                            stop=(j != 2),
                            skip_group_check=True,
                        )
                    nc.scalar.activation(
                        es[hh][:, 0:1408], pss[hh][:, 0:1408], EXPF, scale=SCALE
                    )
                    for j in range(4):
                        o = OFFS[j]
                        nc.vector.tensor_mul(
                            es[hh][:, o:o + P], es[hh][:, o:o + P], dmask[:]
                        )
            return es

        def make_ctx(m2, qb, heads, kind, kts, es, pctx, first_ctx, last):
            def emit():
                for hh in heads:
                    if kind == "full":
                        for i, kt in enumerate(kts):
                            nc.tensor.matmul(
                                pctx[hh][0:HD + 1, :],
                                lhsT=vtm[:, kt, hh, :],
                                rhs=es[hh][:, i * QB:(i + 1) * QB],
                                start=first_ctx[hh],
                                stop=False,
                                skip_group_check=True,
                            )
                            first_ctx[hh] = False
                    else:
                        for j in range(4):
                            nc.tensor.matmul(
                                pctx[hh][0:HD + 1, QOFFS[j]:QB],
                                lhsT=vtm[:, kts[j], hh, :],
                                rhs=es[hh][:, OFFS[j]:OFFS[j] + LENS[j]],
                                start=first_ctx[hh],
                                stop=(j == 3),
                                skip_group_check=True,
                            )
                            first_ctx[hh] = False
                if not last:
                    return
                for hh in heads:
                    hp = (hh % 2) * HD
                    nc.vector.tensor_copy(
                        ctxU[hp:hp + HD, m2, qb * QB:(qb + 1) * QB],
                        pctx[hh][0:HD, :],
                    )
                    nc.vector.tensor_copy(
                        dstage[HD:HD + 1, hh, :], pctx[hh][HD:HD + 1, :]
                    )
                if m2 == HPC // 2 - 1:
                    rc = slice(qb * HPC, (qb + 1) * HPC)
                    nc.sync.dma_start(
                        den2[0:HD, qb, :], dstage[HD:HD + 1, :, :]
                    )
                    nc.vector.reciprocal(rec2[0:HD, qb, :], den2[0:HD, qb, :])
                    nc.vector.tensor_copy(
                        hilo2[0:HD, qb, 0, :], rec2[0:HD, qb, :]
                    )
                    nc.vector.tensor_sub(
                        e1t2[0:HD, qb, :], rec2[0:HD, qb, :],
                        hilo2[0:HD, qb, 0, :],
                    )
                    nc.vector.tensor_copy(
                        hilo2[0:HD, qb, 1, :], e1t2[0:HD, qb, :]
                    )
                    nc.sync.dma_start(
                        rrowbig[HD:HD + 1, rc, :], hilo2[0:HD, qb, 0, :]
                    )
                    nc.sync.dma_start(
                        rrowbig[HD + 1:HD + 2, rc, :], hilo2[0:HD, qb, 1, :]
                    )
            return emit

        pending = None
        for qb in range(NQB):
            for m2 in range(HPC // 2):
                heads = (2 * m2, 2 * m2 + 1)
                pctx = {
                    hh: ctxps.tile([P, QB], F32, tag=f"pctx{i}",
                                   name=f"pctx{i}")
                    for i, hh in enumerate(heads)
                }
                first_ctx = {hh: True for hh in heads}
                fulls = list(range(4 * qb))
                groups = [("full", fulls[i:i + 3])
                          for i in range(0, len(fulls), 3)]
                groups.append(("diag", [4 * qb + j for j in range(4)]))
                for gi, (kind, kts) in enumerate(groups):
                    es = emit_scores_exp(m2, qb, heads, kind, kts)
                    if pending is not None:
                        pending()
                    pending = make_ctx(m2, qb, heads, kind, kts, es,
                                       pctx, first_ctx, gi == len(groups) - 1)
        pending()

    # ---------------- Phase C: normalize + out-project per q-range ----------
    with (
        tc.tile_pool(name="normps", bufs=4, space="PSUM") as normps,
        tc.tile_pool(name="outps", bufs=4, space="PSUM") as outps,
        tc.tile_pool(name="outsb2", bufs=3) as outsb2,
    ):
        def norm_chunk(qb):
            def emit():
                for m2 in range(NQB):
                    bcps = normps.tile([P, QB], F32, tag="bcps")
                    for half in range(2):
                        idx = qb * HPC + 2 * m2 + half
                        nc.tensor.matmul(
                            bcps[half * HD:(half + 1) * HD, :],
                            lhsT=onesbf[HD:HD + 2, :],
                            rhs=rrowbig[HD:HD + 2, idx, :],
                            start=True,
                            stop=True,
                        )
                    nc.vector.tensor_mul(
                        ctxT[:, m2, qb * QB:(qb + 1) * QB],
                        ctxU[:, m2, qb * QB:(qb + 1) * QB],
                        bcps[:],
                    )
            return emit

        def proj_chunk(nts):
            def emit():
                for nt in nts:
                    stage = outsb2.tile([P, D], BF16, tag="stage")
                    for ec in range(D // QB):
                        pso = outps.tile([P, QB], F32, tag="outps",
                                         name=f"outps{ec}")
                        for m in range(FPC // P):
                            nc.tensor.matmul(
                                pso[:],
                                lhsT=ctxT[:, m, nt * P:(nt + 1) * P],
                                rhs=wo_sb[:, m, ec * QB:(ec + 1) * QB],
                                start=(m == 0),
                                stop=(m == FPC // P - 1),
                            )
                        es_ = slice(ec * QB, (ec + 1) * QB)
                        nc.scalar.activation(
                            stage[:, es_], pso[:],
                            mybir.ActivationFunctionType.Copy,
                        )
                        nc.sync.dma_start(out[nt * P:(nt + 1) * P, es_],
                                          stage[:, es_])
            return emit

        for qb in range(NQB):
            norm_chunk(qb)()
            proj_chunk(range(4 * qb, 4 * qb + 4))()

    stageB.release()
    persist.release()


_program_cache = None
last_results = None


def _get_program():
    global _program_cache
    if _program_cache is None:
        _program_cache = _build_program()
    return _program_cache


def kernel(x, Wq, Wk, Wv, Wo, bo):
    global last_results
    x = np.asarray(x, dtype=np.float32)
    Wq = np.asarray(Wq, dtype=np.float32)
    Wk = np.asarray(Wk, dtype=np.float32)
    Wv = np.asarray(Wv, dtype=np.float32)
    Wo = np.asarray(Wo, dtype=np.float32)
    bo = np.asarray(bo, dtype=np.float32)

    bf16 = ml_dtypes.bfloat16
    in_maps = []
    for c in range(NCORES):
        b, g = c // GPC, c % GPC
        fs = slice(g * FPC, (g + 1) * FPC)
        in_maps.append(
            {
                "xT": np.ascontiguousarray(x[b].T).astype(bf16),
                "wq": np.ascontiguousarray(Wq[fs, :].T).astype(bf16),
                "wk": np.ascontiguousarray(Wk[fs, :].T).astype(bf16),
                "wv": np.ascontiguousarray(Wv[fs, :].T).astype(bf16),
                "wo": np.ascontiguousarray(Wo[:, fs].T).astype(bf16),
            }
        )

    nc = _get_program()
    res = run_bass_kernel_spmd(nc, in_maps, core_ids=list(range(NCORES)))
    last_results = res

    outf = np.empty((B, S, D), dtype=np.float32)
    for b in range(B):
        outf[b] = (
            res.results[GPC * b]["out"].astype(np.float32)
            + res.results[GPC * b + 1]["out"].astype(np.float32)
            + bo
        )
    return outf

